# revision 4
# baseline (speedup 1.0000x reference)
"""Causal self-attention (B=2, S=2048, E=1024, H=16, D=64) on 8 TRN2 NeuronCores.

Sharding: core c handles batch b = c//4 and head group hg = c%4 (4 heads).
Each core computes q/k/v projections for its heads, causal attention, and a
row-slice of the output projection; the host sums the 4 partial outputs per
batch and adds b_out.

All matmuls run in float32r (fp32 storage, full-rate PE mode, ~1.5e-4 rounding).
Layouts are transposed so every matmul contraction sits on SBUF partitions:
  qkT   [128, 4, 2048]  col-chunks x tokens (Q heads 01|23, K heads 01|23)
  vaug  [128, 16, 4, 65] per tok-chunk, per head, [v | 1] (ones col -> softmax
                         sums appear as row 64 of the y-matmul output)
  expT  [128, 16, 512]  tok_k x tok_q per (head, q-tile), causal chunks only
  yT    [128, 2, 512]   normalized head outputs packed for the out-projection
  outT  [1024, 2048]    partial (y @ W_out).T
"""

import numpy as np

import concourse.bacc as bacc
import concourse.tile as tile
import concourse.mybir as mybir
from concourse.bass_utils import run_bass_kernel_spmd

B, S, E, H, D = 2, 2048, 1024, 16, 64
NH = 4           # heads per core
EC = NH * D      # 256 embedding cols per core
P = 128
TQ = 512         # q-tile (matmul free dim)
NT = S // TQ     # 4 q-tiles
NKC = S // P     # 16 k-chunks
NE = E // P      # 8 contraction chunks for projections
F32R = mybir.dt.float32r
F32 = mybir.dt.float32
Exp = mybir.ActivationFunctionType.Exp

_prog_cache = {}


def _build():
    nc = bacc.Bacc("TRN2", target_bir_lowering=False, debug=False, num_devices=8)
    XT = nc.dram_tensor("xt", [E, S], F32R, kind="ExternalInput")
    WQK = nc.dram_tensor("wqk", [E, 2 * EC], F32R, kind="ExternalInput")
    WV = nc.dram_tensor("wv", [E, EC], F32R, kind="ExternalInput")
    WO = nc.dram_tensor("wo", [EC, E], F32R, kind="ExternalInput")
    BQK = nc.dram_tensor("bqk", [P, 4], F32, kind="ExternalInput")
    BV = nc.dram_tensor("bv", [P, 2], F32, kind="ExternalInput")
    MSK = nc.dram_tensor("msk", [P, 4, TQ], F32R, kind="ExternalInput")
    OUT = nc.dram_tensor("out", [E, S], F32, kind="ExternalOutput")

    with tile.TileContext(nc) as tc:
        with (
            tc.tile_pool(name="consts", bufs=1) as consts,
            tc.tile_pool(name="qkp", bufs=1) as qkp,
            tc.tile_pool(name="vp", bufs=1) as vp,
            tc.tile_pool(name="ytp", bufs=2) as ytp,
            tc.tile_pool(name="small", bufs=2) as small,
            tc.tile_pool(name="obp", bufs=3) as obp,
            tc.tile_pool(name="pgen", bufs=4, space="PSUM") as pgen,
            tc.tile_pool(name="pscore", bufs=2, space="PSUM") as pscore,
        ):
            # ---- constants ----
            wqk_sb = consts.tile([P, NE, 2 * EC], F32R)
            nc.sync.dma_start(wqk_sb[:], WQK[:].rearrange("(a p) c -> p a c", p=P))
            wv_sb = consts.tile([P, NE, EC], F32R)
            nc.sync.dma_start(wv_sb[:], WV[:].rearrange("(a p) c -> p a c", p=P))
            wo_sb = consts.tile([P, EC // P, E], F32R)
            nc.sync.dma_start(wo_sb[:], WO[:].rearrange("(a p) c -> p a c", p=P))
            bqk_sb = consts.tile([P, 4], F32)
            nc.sync.dma_start(bqk_sb[:], BQK[:])
            bv_sb = consts.tile([P, 2], F32)
            nc.sync.dma_start(bv_sb[:], BV[:])
            msk_sb = consts.tile([P, 4, TQ], F32R)
            nc.sync.dma_start(msk_sb[:], MSK[:])
            ones_f32 = consts.tile([P, 1], F32)
            nc.vector.memset(ones_f32[:], 1.0)
            ones_r = consts.tile([1, D], F32R)
            nc.vector.tensor_copy(ones_r[:], ones_f32[0:1, :].to_broadcast((1, D)))

            qkT = qkp.tile([P, 4, S], F32R)
            vaug = vp.tile([P, NKC, NH, D + 1], F32R)
            nc.vector.tensor_copy(
                vaug[:, :, :, D : D + 1], ones_f32[:].to_broadcast((P, NKC, NH, 1))
            )

            # ---- phase A: projections (xT scoped so its SBUF is reused) ----
            with tc.tile_pool(name="xp", bufs=1) as xp:
                xT = xp.tile([P, NE, S], F32R)
                XTr = XT[:].rearrange("(a p) t -> p a t", p=P)
                for e in range(NE):
                    nc.sync.dma_start(xT[:, e, :], XTr[:, e, :])

                # V projection: out rows = tokens
                for c in range(NKC):
                    pv = pgen.tile([P, TQ], F32, tag="mm")
                    for e in range(NE):
                        nc.tensor.matmul(
                            pv[:, 0:EC],
                            xT[:, e, c * P : (c + 1) * P],
                            wv_sb[:, e, :],
                            start=(e == 0),
                            stop=(e == NE - 1),
                        )
                    nc.scalar.copy(
                        vaug[:, c, :, 0:D],
                        pv[:, 0:EC].rearrange("p (h d) -> p h d", d=D),
                    )

                # QK projection: out rows = q/k columns
                for cc in range(4):
                    for t in range(NT):
                        pq = pgen.tile([P, TQ], F32, tag="mm")
                        for e in range(NE):
                            nc.tensor.matmul(
                                pq[:],
                                wqk_sb[:, e, cc * P : (cc + 1) * P],
                                xT[:, e, t * TQ : (t + 1) * TQ],
                                start=(e == 0),
                                stop=(e == NE - 1),
                            )
                        nc.vector.tensor_scalar_add(
                            qkT[:, cc, t * TQ : (t + 1) * TQ],
                            pq[:],
                            bqk_sb[:, cc : cc + 1],
                        )

            # ---- phases B+C: attention + out-projection ----
            with tc.tile_pool(name="etp", bufs=2) as etp:
                for t in range(NT):
                    yT = ytp.tile([P, EC // P, TQ], F32R)
                    for h in range(NH):
                        pbase = D * (h % 2)
                        qi = h // 2
                        ki = 2 + h // 2
                        nchunks = 4 * (t + 1)
                        et = etp.tile([P, NKC, TQ], F32R)
                        qT = qkT[pbase : pbase + D, qi, t * TQ : (t + 1) * TQ]
                        for g in range(2 * (t + 1)):
                            psc = pscore.tile([P, 2 * TQ], F32, tag="sc")
                            for j in range(2):
                                c = 2 * g + j
                                nc.tensor.matmul(
                                    psc[:, j * TQ : (j + 1) * TQ],
                                    qkT[pbase : pbase + D, ki, c * P : (c + 1) * P],
                                    qT,
                                    start=True,
                                    stop=True,
                                )
                            nc.scalar.activation(
                                et[:, 2 * g : 2 * g + 2, :],
                                psc[:].rearrange("p (a q) -> p a q", a=2),
                                Exp,
                                scale=float(1.0 / np.sqrt(D)),
                            )
                            if g >= 2 * t:  # diagonal chunks: causal mask
                                jd = 2 * (g - 2 * t)
                                nc.vector.tensor_tensor(
                                    et[:, 2 * g : 2 * g + 2, :],
                                    et[:, 2 * g : 2 * g + 2, :],
                                    msk_sb[:, jd : jd + 2, :],
                                    mybir.AluOpType.mult,
                                )
                        # y = v_aug.T @ expT (row 64 = softmax sums)
                        pyt = pgen.tile([P, TQ], F32, tag="mm")
                        for c in range(nchunks):
                            nc.tensor.matmul(
                                pyt[0 : D + 1, :],
                                vaug[:, c, h, :],
                                et[:, c, :],
                                start=(c == 0),
                                stop=(c == nchunks - 1),
                            )
                        rc_s = small.tile([1, TQ], F32)
                        nc.vector.tensor_copy(rc_s[:], pyt[D : D + 1, :])
                        rc_f = small.tile([1, TQ], F32)
                        nc.vector.reciprocal(rc_f[:], rc_s[:])
                        rc_r = small.tile([1, TQ], F32R)
                        nc.vector.tensor_copy(rc_r[:], rc_f[:])
                        pb = pgen.tile([P, TQ], F32, tag="mm")
                        nc.tensor.matmul(
                            pb[0:D, :], ones_r[:], rc_r[:], start=True, stop=True
                        )
                        pbs = small.tile([D, TQ], F32)
                        nc.vector.tensor_copy(pbs[:], pb[0:D, :])
                        yslot = yT[pbase : pbase + D, h // 2, :]
                        nc.vector.tensor_tensor(
                            yslot, pyt[0:D, :], pbs[:], mybir.AluOpType.mult
                        )
                        nc.vector.tensor_scalar_add(
                            yslot, yslot, bv_sb[pbase : pbase + D, h // 2 : h // 2 + 1]
                        )

                    # out-projection for this q-tile
                    for eo in range(NE):
                        po = pgen.tile([P, TQ], F32, tag="mm")
                        for a in range(EC // P):
                            nc.tensor.matmul(
                                po[:],
                                wo_sb[:, a, eo * P : (eo + 1) * P],
                                yT[:, a, :],
                                start=(a == 0),
                                stop=(a == EC // P - 1),
                            )
                        ot = obp.tile([P, TQ], F32)
                        nc.vector.tensor_copy(ot[:], po[:])
                        nc.sync.dma_start(
                            OUT[eo * P : (eo + 1) * P, t * TQ : (t + 1) * TQ], ot[:]
                        )

    nc.compile()
    return nc


def _shard_inputs(x, W_qkv, b_qkv, W_out, b_out):
    """Build the 8 per-core input maps (host-side layout preprocessing)."""
    Wq, Wk, Wv = W_qkv[:, 0:E], W_qkv[:, E : 2 * E], W_qkv[:, 2 * E : 3 * E]
    bq, bk, bv = b_qkv[0:E], b_qkv[E : 2 * E], b_qkv[2 * E : 3 * E]

    # causal mask for the 4 diagonal 128-chunks of a 512-wide q-tile
    p = np.arange(P)[:, None, None]
    j = np.arange(4)[None, :, None]
    f = np.arange(TQ)[None, None, :]
    msk = (p + P * j <= f).astype(np.float32)

    in_maps = []
    for c in range(8):
        b, hg = c // 4, c % 4
        cs = slice(hg * EC, (hg + 1) * EC)
        in_maps.append(
            {
                "xt": np.ascontiguousarray(x[b].T),
                "wqk": np.ascontiguousarray(
                    np.concatenate([Wq[:, cs], Wk[:, cs]], axis=1)
                ),
                "wv": np.ascontiguousarray(Wv[:, cs]),
                "wo": np.ascontiguousarray(W_out[cs, :]),
                "bqk": np.ascontiguousarray(
                    np.concatenate([bq[cs], bk[cs]]).reshape(4, P).T
                ),
                "bv": np.ascontiguousarray(bv[cs].reshape(2, P).T),
                "msk": msk,
            }
        )
    return in_maps


def _run(inputs, trace=False):
    x = np.asarray(inputs["x"], dtype=np.float32)
    W_qkv = np.asarray(inputs["W_qkv"], dtype=np.float32)
    b_qkv = np.asarray(inputs["b_qkv"], dtype=np.float32)
    W_out = np.asarray(inputs["W_out"], dtype=np.float32)
    b_out = np.asarray(inputs["b_out"], dtype=np.float32)

    if "prog" not in _prog_cache:
        _prog_cache["prog"] = _build()
    nc = _prog_cache["prog"]

    in_maps = _shard_inputs(x, W_qkv, b_qkv, W_out, b_out)
    res = run_bass_kernel_spmd(nc, in_maps, core_ids=list(range(8)), trace=trace)

    out = np.zeros((B, S, E), dtype=np.float64)
    for c in range(8):
        out[c // 4] += res.results[c]["out"].astype(np.float64).T
    out += b_out.astype(np.float64)
    return out.astype(np.float32), res


def kernel(**inputs) -> np.ndarray:
    y, _ = _run(inputs, trace=False)
    return y


# revision 5
# speedup vs baseline: 1.5183x; 1.5183x over previous
"""Causal self-attention (B=2, S=2048, E=1024, H=16, D=64) on 8 TRN2 NeuronCores.

Sharding: core c handles batch b = c//4 and head group hg = c%4 (4 heads).
Each core computes q/k/v projections for its heads, causal attention, and a
row-slice of the output projection; the host sums the 4 partial outputs per
batch and adds b_out.

Matmul operands are fp16 (full-rate PE + fast weight load); accumulation is
fp32 in PSUM, softmax stays fp32 until the weights are written. Layouts put
every contraction on SBUF partitions:
  qkT   [128, 4, 2048]   col-chunks x tokens (Q heads 01|23, K heads 01|23)
  vaug  [128, 16, 4, 65] per tok-chunk, per head, [v | 1] (ones col -> softmax
                         sums appear as row 64 of the y-matmul output)
  expT  [128, 16, 512]   tok_k x tok_q per (head, q-tile), causal chunks only
  yT    [128, 2, 512]    normalized head outputs packed for the out-projection
  outT  [1024, 2048]     partial (y @ W_out).T

The per-head softmax normalization (sum row -> reciprocal -> PE broadcast ->
scale) is software-pipelined one head behind the matmul stream so the PE never
idles long enough to re-throttle (HAM).
"""

import numpy as np

import concourse.bacc as bacc
import concourse.tile as tile
import concourse.mybir as mybir
from concourse.bass_utils import run_bass_kernel_spmd

B, S, E, H, D = 2, 2048, 1024, 16, 64
NH = 4           # heads per core
EC = NH * D      # 256 embedding cols per core
P = 128
TQ = 512         # q-tile (matmul free dim)
NT = S // TQ     # 4 q-tiles
NKC = S // P     # 16 k-chunks
NE = E // P      # 8 contraction chunks for projections
F16 = mybir.dt.float16
F32 = mybir.dt.float32
Exp = mybir.ActivationFunctionType.Exp

_prog_cache = {}


def _build():
    nc = bacc.Bacc("TRN2", target_bir_lowering=False, debug=False, num_devices=8)
    XT = nc.dram_tensor("xt", [E, S], F16, kind="ExternalInput")
    WQK = nc.dram_tensor("wqk", [E, 2 * EC], F16, kind="ExternalInput")
    WV = nc.dram_tensor("wv", [E, EC], F16, kind="ExternalInput")
    WO = nc.dram_tensor("wo", [EC, E], F16, kind="ExternalInput")
    BQK = nc.dram_tensor("bqk", [P, 4], F32, kind="ExternalInput")
    BV = nc.dram_tensor("bv", [P, 2], F32, kind="ExternalInput")
    MSK = nc.dram_tensor("msk", [P, 4, TQ], F16, kind="ExternalInput")
    OUT = nc.dram_tensor("out", [E, S], F32, kind="ExternalOutput")

    with tile.TileContext(nc) as tc:
        with (
            tc.tile_pool(name="consts", bufs=1) as consts,
            tc.tile_pool(name="qkp", bufs=1) as qkp,
            tc.tile_pool(name="vp", bufs=1) as vp,
            tc.tile_pool(name="ytp", bufs=2) as ytp,
            tc.tile_pool(name="small", bufs=3) as small,
            tc.tile_pool(name="obp", bufs=3) as obp,
            tc.tile_pool(name="pgen", bufs=4, space="PSUM") as pgen,
            tc.tile_pool(name="pscore", bufs=2, space="PSUM") as pscore,
        ):
            # ---- constants ----
            wqk_sb = consts.tile([P, NE, 2 * EC], F16)
            nc.sync.dma_start(wqk_sb[:], WQK[:].rearrange("(a p) c -> p a c", p=P))
            wv_sb = consts.tile([P, NE, EC], F16)
            nc.sync.dma_start(wv_sb[:], WV[:].rearrange("(a p) c -> p a c", p=P))
            wo_sb = consts.tile([P, EC // P, E], F16)
            nc.sync.dma_start(wo_sb[:], WO[:].rearrange("(a p) c -> p a c", p=P))
            bqk_sb = consts.tile([P, 4], F32)
            nc.sync.dma_start(bqk_sb[:], BQK[:])
            bv_sb = consts.tile([P, 2], F32)
            nc.sync.dma_start(bv_sb[:], BV[:])
            msk_sb = consts.tile([P, 4, TQ], F16)
            nc.sync.dma_start(msk_sb[:], MSK[:])
            ones_f32 = consts.tile([P, 1], F32)
            nc.vector.memset(ones_f32[:], 1.0)
            ones_16 = consts.tile([1, D], F16)
            nc.vector.tensor_copy(ones_16[:], ones_f32[0:1, :].to_broadcast((1, D)))

            qkT = qkp.tile([P, 4, S], F16)
            vaug = vp.tile([P, NKC, NH, D + 1], F16)
            nc.vector.tensor_copy(
                vaug[:, :, :, D : D + 1], ones_f32[:].to_broadcast((P, NKC, NH, 1))
            )

            # ---- phase A: projections (xT scoped so its SBUF is reused) ----
            with tc.tile_pool(name="xp", bufs=1) as xp:
                xT = xp.tile([P, NE, S], F16)
                XTr = XT[:].rearrange("(a p) t -> p a t", p=P)
                for e in range(NE):
                    nc.sync.dma_start(xT[:, e, :], XTr[:, e, :])

                # V projection: out rows = tokens
                for c in range(NKC):
                    pv = pgen.tile([P, TQ], F32, tag="mm")
                    for e in range(NE):
                        nc.tensor.matmul(
                            pv[:, 0:EC],
                            xT[:, e, c * P : (c + 1) * P],
                            wv_sb[:, e, :],
                            start=(e == 0),
                            stop=(e == NE - 1),
                        )
                    nc.scalar.copy(
                        vaug[:, c, :, 0:D],
                        pv[:, 0:EC].rearrange("p (h d) -> p h d", d=D),
                    )

                # QK projection: out rows = q/k columns
                for cc in range(4):
                    for t in range(NT):
                        pq = pgen.tile([P, TQ], F32, tag="mm")
                        for e in range(NE):
                            nc.tensor.matmul(
                                pq[:],
                                wqk_sb[:, e, cc * P : (cc + 1) * P],
                                xT[:, e, t * TQ : (t + 1) * TQ],
                                start=(e == 0),
                                stop=(e == NE - 1),
                            )
                        nc.vector.tensor_scalar_add(
                            qkT[:, cc, t * TQ : (t + 1) * TQ],
                            pq[:],
                            bqk_sb[:, cc : cc + 1],
                        )

            # ---- phases B+C: attention + out-projection ----
            with tc.tile_pool(name="etp", bufs=2) as etp:

                def normalize(st):
                    """Flush the deferred normalization of a finished head."""
                    pyt, rc_r, yslot, pb_lo = st
                    pb = pgen.tile([P, TQ], F32, tag="mm")
                    nc.tensor.matmul(
                        pb[0:D, :], ones_16[:], rc_r[:], start=True, stop=True
                    )
                    pbs = small.tile([D, TQ], F32)
                    nc.vector.tensor_copy(pbs[:], pb[0:D, :])
                    nc.vector.tensor_tensor(
                        yslot, pyt[0:D, :], pbs[:], mybir.AluOpType.mult
                    )
                    nc.vector.tensor_scalar_add(yslot, yslot, pb_lo)

                for t in range(NT):
                    yT = ytp.tile([P, EC // P, TQ], F16)
                    pending = None
                    for h in range(NH):
                        pbase = D * (h % 2)
                        qi = h // 2
                        ki = 2 + h // 2
                        nchunks = 4 * (t + 1)
                        et = etp.tile([P, NKC, TQ], F16)
                        qT = qkT[pbase : pbase + D, qi, t * TQ : (t + 1) * TQ]
                        for g in range(2 * (t + 1)):
                            psc = pscore.tile([P, 2 * TQ], F32, tag="sc")
                            for j in range(2):
                                c = 2 * g + j
                                nc.tensor.matmul(
                                    psc[:, j * TQ : (j + 1) * TQ],
                                    qkT[pbase : pbase + D, ki, c * P : (c + 1) * P],
                                    qT,
                                    start=True,
                                    stop=True,
                                )
                            nc.scalar.activation(
                                et[:, 2 * g : 2 * g + 2, :],
                                psc[:].rearrange("p (a q) -> p a q", a=2),
                                Exp,
                                scale=float(1.0 / np.sqrt(D)),
                            )
                            if g >= 2 * t:  # diagonal chunks: causal mask
                                jd = 2 * (g - 2 * t)
                                nc.vector.tensor_tensor(
                                    et[:, 2 * g : 2 * g + 2, :],
                                    et[:, 2 * g : 2 * g + 2, :],
                                    msk_sb[:, jd : jd + 2, :],
                                    mybir.AluOpType.mult,
                                )
                        # y = v_aug.T @ expT (row 64 = softmax sums)
                        pyt = pgen.tile([P, TQ], F32, tag="mm")
                        for c in range(nchunks):
                            nc.tensor.matmul(
                                pyt[0 : D + 1, :],
                                vaug[:, c, h, :],
                                et[:, c, :],
                                start=(c == 0),
                                stop=(c == nchunks - 1),
                            )
                        # reciprocal of sums (DVE, overlaps next head's matmuls)
                        rc_s = small.tile([1, TQ], F32)
                        nc.vector.tensor_copy(rc_s[:], pyt[D : D + 1, :])
                        rc_f = small.tile([1, TQ], F32)
                        nc.vector.reciprocal_approx_fast(rc_f[:], rc_s[:])
                        rc_r = small.tile([1, TQ], F16)
                        nc.vector.tensor_copy(rc_r[:], rc_f[:])
                        if pending is not None:
                            normalize(pending)
                        yslot = yT[pbase : pbase + D, h // 2, :]
                        pending = (
                            pyt,
                            rc_r,
                            yslot,
                            bv_sb[pbase : pbase + D, h // 2 : h // 2 + 1],
                        )
                    normalize(pending)

                    # out-projection for this q-tile
                    for eo in range(NE):
                        po = pgen.tile([P, TQ], F32, tag="mm")
                        for a in range(EC // P):
                            nc.tensor.matmul(
                                po[:],
                                wo_sb[:, a, eo * P : (eo + 1) * P],
                                yT[:, a, :],
                                start=(a == 0),
                                stop=(a == EC // P - 1),
                            )
                        ot = obp.tile([P, TQ], F32)
                        nc.vector.tensor_copy(ot[:], po[:])
                        nc.sync.dma_start(
                            OUT[eo * P : (eo + 1) * P, t * TQ : (t + 1) * TQ], ot[:]
                        )

    nc.compile()
    return nc


def _shard_inputs(x, W_qkv, b_qkv, W_out, b_out):
    """Build the 8 per-core input maps (host-side layout preprocessing)."""
    Wq, Wk, Wv = W_qkv[:, 0:E], W_qkv[:, E : 2 * E], W_qkv[:, 2 * E : 3 * E]
    bq, bk, bv = b_qkv[0:E], b_qkv[E : 2 * E], b_qkv[2 * E : 3 * E]

    # causal mask for the 4 diagonal 128-chunks of a 512-wide q-tile
    p = np.arange(P)[:, None, None]
    j = np.arange(4)[None, :, None]
    f = np.arange(TQ)[None, None, :]
    msk = (p + P * j <= f).astype(np.float16)

    in_maps = []
    for c in range(8):
        b, hg = c // 4, c % 4
        cs = slice(hg * EC, (hg + 1) * EC)
        in_maps.append(
            {
                "xt": np.ascontiguousarray(x[b].T.astype(np.float16)),
                "wqk": np.ascontiguousarray(
                    np.concatenate([Wq[:, cs], Wk[:, cs]], axis=1).astype(np.float16)
                ),
                "wv": np.ascontiguousarray(Wv[:, cs].astype(np.float16)),
                "wo": np.ascontiguousarray(W_out[cs, :].astype(np.float16)),
                "bqk": np.ascontiguousarray(
                    np.concatenate([bq[cs], bk[cs]]).reshape(4, P).T
                ),
                "bv": np.ascontiguousarray(bv[cs].reshape(2, P).T),
                "msk": msk,
            }
        )
    return in_maps


def _run(inputs, trace=False):
    x = np.asarray(inputs["x"], dtype=np.float32)
    W_qkv = np.asarray(inputs["W_qkv"], dtype=np.float32)
    b_qkv = np.asarray(inputs["b_qkv"], dtype=np.float32)
    W_out = np.asarray(inputs["W_out"], dtype=np.float32)
    b_out = np.asarray(inputs["b_out"], dtype=np.float32)

    if "prog" not in _prog_cache:
        _prog_cache["prog"] = _build()
    nc = _prog_cache["prog"]

    in_maps = _shard_inputs(x, W_qkv, b_qkv, W_out, b_out)
    res = run_bass_kernel_spmd(nc, in_maps, core_ids=list(range(8)), trace=trace)

    out = np.zeros((B, S, E), dtype=np.float64)
    for c in range(8):
        out[c // 4] += res.results[c]["out"].astype(np.float64).T
    out += b_out.astype(np.float64)
    return out.astype(np.float32), res


def kernel(**inputs) -> np.ndarray:
    y, _ = _run(inputs, trace=False)
    return y


# revision 8
# speedup vs baseline: 1.5487x; 1.0200x over previous
"""Causal self-attention (B=2, S=2048, E=1024, H=16, D=64) on 8 TRN2 NeuronCores.

Sharding: core c handles batch b = c//4 and head group hg = c%4 (4 heads).
Each core computes q/k/v projections for its heads, causal attention, and a
row-slice of the output projection; the host sums the 4 partial outputs per
batch and adds b_out.

Matmul operands are fp16 (full-rate PE + fast weight load); accumulation is
fp32 in PSUM. Layouts put every contraction on SBUF partitions:
  qkT   [128, 4, 2048]   Q cols (head pairs 01|23) then K cols, x tokens
  vaug  [128, 16, 4, 65] per tok-chunk, per head, [v | 1] (ones col -> softmax
                         sums appear as row 64 of the y-matmul output)
  yT    [128, 2, 512]    normalized head outputs packed for the out-projection
  outT  [1024, 2048]     partial (y @ W_out).T

Attention is pipelined per 128-token k-chunk: the two heads of a pair are
packed into one [128,1024] scores PSUM tile (even head -> PE rows 0-63, odd
head -> rows 64-127, running concurrently), exp'd in one ScalarE call, then
immediately consumed by the y-matmuls. Softmax normalization, out-projection
and the remaining QK projection tiles ride a deferred-work queue that drains
one item per chunk into the PE stream's ACT-wait gaps, keeping the PE dense
(and the HAM clock warm).
"""

import numpy as np

import concourse.bacc as bacc
import concourse.tile as tile
import concourse.mybir as mybir
from concourse.bass_utils import run_bass_kernel_spmd

B, S, E, H, D = 2, 2048, 1024, 16, 64
NH = 4           # heads per core
EC = NH * D      # 256 embedding cols per core
P = 128
TQ = 512         # q-tile (matmul free dim)
NT = S // TQ     # 4 q-tiles
NKC = S // P     # 16 k-chunks
NE = E // P      # 8 contraction chunks for projections
F16 = mybir.dt.float16
F32 = mybir.dt.float32
Exp = mybir.ActivationFunctionType.Exp
SCALE = float(1.0 / np.sqrt(D))

_prog_cache = {}


def _build():
    nc = bacc.Bacc("TRN2", target_bir_lowering=False, debug=False, num_devices=8)
    XT = nc.dram_tensor("xt", [E, S], F16, kind="ExternalInput")
    WQK = nc.dram_tensor("wqk", [E, 2 * EC], F16, kind="ExternalInput")
    WV = nc.dram_tensor("wv", [E, EC], F16, kind="ExternalInput")
    WO = nc.dram_tensor("wo", [EC, E], F16, kind="ExternalInput")
    BQK = nc.dram_tensor("bqk", [P, 4], F32, kind="ExternalInput")
    BV = nc.dram_tensor("bv", [P, 2], F32, kind="ExternalInput")
    MSK = nc.dram_tensor("msk", [P, 4, TQ], F16, kind="ExternalInput")
    OUT = nc.dram_tensor("out", [E, S], F32, kind="ExternalOutput")

    with tile.TileContext(nc) as tc:
        with (
            tc.tile_pool(name="consts", bufs=1) as consts,
            tc.tile_pool(name="qkp", bufs=1) as qkp,
            tc.tile_pool(name="vp", bufs=1) as vp,
            tc.tile_pool(name="xp", bufs=1) as xp,
            tc.tile_pool(name="ytp", bufs=2) as ytp,
            tc.tile_pool(name="small", bufs=4) as small,
            tc.tile_pool(name="obp", bufs=3) as obp,
            tc.tile_pool(name="etp", bufs=4) as etp,
            tc.tile_pool(name="pgen", bufs=4, space="PSUM") as pgen,
            tc.tile_pool(name="pscore", bufs=2, space="PSUM") as pscore,
        ):
            # ---- constants ----
            wqk_sb = consts.tile([P, NE, 2 * EC], F16)
            nc.sync.dma_start(wqk_sb[:], WQK[:].rearrange("(a p) c -> p a c", p=P))
            wv_sb = consts.tile([P, NE, EC], F16)
            nc.sync.dma_start(wv_sb[:], WV[:].rearrange("(a p) c -> p a c", p=P))
            wo_sb = consts.tile([P, EC // P, E], F16)
            nc.sync.dma_start(wo_sb[:], WO[:].rearrange("(a p) c -> p a c", p=P))
            bqk_sb = consts.tile([P, 4], F32)
            nc.sync.dma_start(bqk_sb[:], BQK[:])
            bv_sb = consts.tile([P, 2], F32)
            nc.sync.dma_start(bv_sb[:], BV[:])
            msk_sb = consts.tile([P, 4, TQ], F16)
            nc.sync.dma_start(msk_sb[:], MSK[:])
            ones_f32 = consts.tile([P, 1], F32)
            nc.vector.memset(ones_f32[:], 1.0)
            ones_16 = consts.tile([1, D], F16)
            nc.vector.tensor_copy(ones_16[:], ones_f32[0:1, :].to_broadcast((1, D)))

            qkT = qkp.tile([P, 4, S], F16)
            vaug = vp.tile([P, NKC, NH, D + 1], F16)
            nc.vector.tensor_copy(
                vaug[:, :, :, D : D + 1], ones_f32[:].to_broadcast((P, NKC, NH, 1))
            )

            xT = xp.tile([P, NE, S], F16)
            XTr = XT[:].rearrange("(a p) t -> p a t", p=P)
            for e in range(NE):
                nc.sync.dma_start(xT[:, e, :], XTr[:, e, :])

            # ---- emit helpers ----
            def emit_qk_proj(tt, cc):
                pq = pgen.tile([P, TQ], F32, tag="mm")
                for e in range(NE):
                    nc.tensor.matmul(
                        pq[:],
                        wqk_sb[:, e, cc * P : (cc + 1) * P],
                        xT[:, e, tt * TQ : (tt + 1) * TQ],
                        start=(e == 0),
                        stop=(e == NE - 1),
                    )
                nc.vector.tensor_scalar_add(
                    qkT[:, cc, tt * TQ : (tt + 1) * TQ], pq[:], bqk_sb[:, cc : cc + 1]
                )

            def emit_flush(pyt, rc_r, yslot, bv_ap):
                pb = pgen.tile([P, TQ], F32, tag="mm")
                nc.tensor.matmul(
                    pb[0:D, :], ones_16[:], rc_r[:], start=True, stop=True
                )
                pbs = small.tile([D, TQ], F32)
                nc.vector.tensor_copy(pbs[:], pb[0:D, :])
                nc.vector.tensor_tensor(
                    yslot, pyt[0:D, :], pbs[:], mybir.AluOpType.mult
                )
                nc.vector.tensor_scalar_add(yslot, yslot, bv_ap)

            def emit_outproj(yT, t, eo):
                po = pgen.tile([P, TQ], F32, tag="mm")
                for a in range(EC // P):
                    nc.tensor.matmul(
                        po[:],
                        wo_sb[:, a, eo * P : (eo + 1) * P],
                        yT[:, a, :],
                        start=(a == 0),
                        stop=(a == EC // P - 1),
                    )
                ot = obp.tile([P, TQ], F32)
                nc.vector.tensor_copy(ot[:], po[:])
                nc.sync.dma_start(
                    OUT[eo * P : (eo + 1) * P, t * TQ : (t + 1) * TQ], ot[:]
                )

            from collections import deque

            flush_q = deque()  # normalizations: free pyt PSUM slots, run first
            work_q = deque()   # projection tiles / out-projection tiles

            def pop_deferred(n=1):
                for _ in range(n):
                    if flush_q:
                        flush_q.popleft()()
                    elif work_q:
                        work_q.popleft()()

            def drain_flushes():
                while flush_q:
                    flush_q.popleft()()

            # ---- V projection (row-major, ones column appended) ----
            for c in range(NKC):
                pv = pgen.tile([P, TQ], F32, tag="mm")
                for e in range(NE):
                    nc.tensor.matmul(
                        pv[:, 0:EC],
                        xT[:, e, c * P : (c + 1) * P],
                        wv_sb[:, e, :],
                        start=(e == 0),
                        stop=(e == NE - 1),
                    )
                nc.scalar.copy(
                    vaug[:, c, :, 0:D],
                    pv[:, 0:EC].rearrange("p (h d) -> p h d", d=D),
                )

            # ---- QK projection for t=0; the rest ride the deferred queue ----
            for cc in range(4):
                emit_qk_proj(0, cc)

            # ---- attention, chunk-pipelined, head-pair packed ----
            for t in range(NT):
                if t + 1 < NT:
                    for cc in range(4):
                        work_q.append(lambda tt=t + 1, cc=cc: emit_qk_proj(tt, cc))
                yT = ytp.tile([P, EC // P, TQ], F16)
                for pair in range(2):
                    nchunks = 4 * (t + 1)
                    drain_flushes()  # free pending pyt slots before new allocs
                    pyt_he = pgen.tile([P, TQ], F32, tag="mm")
                    pyt_ho = pgen.tile([P, TQ], F32, tag="mm")
                    ets = []
                    for c in range(nchunks):
                        psc = pscore.tile([P, 2 * TQ], F32, tag="sc")
                        nc.tensor.matmul(
                            psc[:, 0:TQ],
                            qkT[0:D, 2 + pair, c * P : (c + 1) * P],
                            qkT[0:D, pair, t * TQ : (t + 1) * TQ],
                            start=True,
                            stop=True,
                        )
                        nc.tensor.matmul(
                            psc[:, TQ : 2 * TQ],
                            qkT[D:P, 2 + pair, c * P : (c + 1) * P],
                            qkT[D:P, pair, t * TQ : (t + 1) * TQ],
                            start=True,
                            stop=True,
                        )
                        et = etp.tile([P, 2, TQ], F16, tag="et")
                        nc.scalar.activation(
                            et[:],
                            psc[:].rearrange("p (a q) -> p a q", a=2),
                            Exp,
                            scale=SCALE,
                        )
                        jd = c - 4 * t
                        if jd >= 0:  # diagonal chunk: causal mask
                            nc.vector.tensor_tensor(
                                et[:],
                                et[:],
                                msk_sb[:, jd : jd + 1, :].to_broadcast((P, 2, TQ)),
                                mybir.AluOpType.mult,
                            )
                        nc.tensor.matmul(
                            pyt_he[0 : D + 1, :],
                            vaug[:, c, 2 * pair, :],
                            et[:, 0, :],
                            start=(c == 0),
                            stop=(c == nchunks - 1),
                        )
                        nc.tensor.matmul(
                            pyt_ho[0 : D + 1, :],
                            vaug[:, c, 2 * pair + 1, :],
                            et[:, 1, :],
                            start=(c == 0),
                            stop=(c == nchunks - 1),
                        )
                        pop_deferred()
                    # reciprocals of the softmax sums (DVE, off the PE path)
                    for idx, pyt in ((0, pyt_he), (1, pyt_ho)):
                        h = 2 * pair + idx
                        pbase = D * idx
                        rc_s = small.tile([1, TQ], F32)
                        nc.vector.tensor_copy(rc_s[:], pyt[D : D + 1, :])
                        rc_f = small.tile([1, TQ], F32)
                        nc.vector.reciprocal_approx_fast(rc_f[:], rc_s[:])
                        rc_r = small.tile([1, TQ], F16)
                        nc.vector.tensor_copy(rc_r[:], rc_f[:])
                        yslot = yT[pbase : pbase + D, pair, :]
                        bv_ap = bv_sb[pbase : pbase + D, pair : pair + 1]
                        flush_q.append(
                            lambda pyt=pyt, rc_r=rc_r, yslot=yslot, bv_ap=bv_ap: (
                                emit_flush(pyt, rc_r, yslot, bv_ap)
                            )
                        )
                    pop_deferred()
                # out-projection for this q-tile rides the queue too
                for eo in range(NE):
                    work_q.append(lambda yT=yT, t=t, eo=eo: emit_outproj(yT, t, eo))
                pop_deferred(2)
            pop_deferred(len(flush_q) + len(work_q))

    nc.compile()
    return nc


def _shard_inputs(x, W_qkv, b_qkv, W_out, b_out):
    """Build the 8 per-core input maps (host-side layout preprocessing)."""
    Wq, Wk, Wv = W_qkv[:, 0:E], W_qkv[:, E : 2 * E], W_qkv[:, 2 * E : 3 * E]
    bq, bk, bv = b_qkv[0:E], b_qkv[E : 2 * E], b_qkv[2 * E : 3 * E]

    # causal mask for the 4 diagonal 128-chunks of a 512-wide q-tile
    p = np.arange(P)[:, None, None]
    j = np.arange(4)[None, :, None]
    f = np.arange(TQ)[None, None, :]
    msk = (p + P * j <= f).astype(np.float16)

    in_maps = []
    for c in range(8):
        b, hg = c // 4, c % 4
        cs = slice(hg * EC, (hg + 1) * EC)
        in_maps.append(
            {
                "xt": np.ascontiguousarray(x[b].T.astype(np.float16)),
                "wqk": np.ascontiguousarray(
                    np.concatenate([Wq[:, cs], Wk[:, cs]], axis=1).astype(np.float16)
                ),
                "wv": np.ascontiguousarray(Wv[:, cs].astype(np.float16)),
                "wo": np.ascontiguousarray(W_out[cs, :].astype(np.float16)),
                "bqk": np.ascontiguousarray(
                    np.concatenate([bq[cs], bk[cs]]).reshape(4, P).T
                ),
                "bv": np.ascontiguousarray(bv[cs].reshape(2, P).T),
                "msk": msk,
            }
        )
    return in_maps


def _run(inputs, trace=False):
    x = np.asarray(inputs["x"], dtype=np.float32)
    W_qkv = np.asarray(inputs["W_qkv"], dtype=np.float32)
    b_qkv = np.asarray(inputs["b_qkv"], dtype=np.float32)
    W_out = np.asarray(inputs["W_out"], dtype=np.float32)
    b_out = np.asarray(inputs["b_out"], dtype=np.float32)

    if "prog" not in _prog_cache:
        _prog_cache["prog"] = _build()
    nc = _prog_cache["prog"]

    in_maps = _shard_inputs(x, W_qkv, b_qkv, W_out, b_out)
    res = run_bass_kernel_spmd(nc, in_maps, core_ids=list(range(8)), trace=trace)

    out = np.zeros((B, S, E), dtype=np.float64)
    for c in range(8):
        out[c // 4] += res.results[c]["out"].astype(np.float64).T
    out += b_out.astype(np.float64)
    return out.astype(np.float32), res


def kernel(**inputs) -> np.ndarray:
    y, _ = _run(inputs, trace=False)
    return y


# revision 10
# speedup vs baseline: 1.5588x; 1.0066x over previous
"""Causal self-attention (B=2, S=2048, E=1024, H=16, D=64) on 8 TRN2 NeuronCores.

Sharding: core c handles batch b = c//4 and head group hg = c%4 (4 heads).
Each core computes q/k/v projections for its heads, causal attention, and a
row-slice of the output projection; the host sums the 4 partial outputs per
batch and adds b_out.

Matmul operands are fp16 (full-rate PE + fast weight load); accumulation is
fp32 in PSUM. Layouts put every contraction on SBUF partitions:
  qkT   [128, 4, 2048]   Q cols (head pairs 01|23) then K cols, x tokens
  vaug  [128, 16, 4, 65] per tok-chunk, per head, [v | 1] (ones col -> softmax
                         sums appear as row 64 of the y-matmul output)
  yT    [128, 2, 512]    normalized head outputs packed for the out-projection
  outT  [1024, 2048]     partial (y @ W_out).T

Attention is pipelined per 128-token k-chunk: the two heads of a pair are
packed into one [128,1024] scores PSUM tile (even head -> PE rows 0-63, odd
head -> rows 64-127, running concurrently), exp'd in one ScalarE call, then
immediately consumed by the y-matmuls. Softmax normalization, out-projection
and the remaining QK projection tiles ride a deferred-work queue that drains
one item per chunk into the PE stream's ACT-wait gaps, keeping the PE dense
(and the HAM clock warm).
"""

import numpy as np

import concourse.bacc as bacc
import concourse.tile as tile
import concourse.mybir as mybir
from concourse.bass_utils import run_bass_kernel_spmd

B, S, E, H, D = 2, 2048, 1024, 16, 64
NH = 4           # heads per core
EC = NH * D      # 256 embedding cols per core
P = 128
TQ = 512         # q-tile (matmul free dim)
NT = S // TQ     # 4 q-tiles
NKC = S // P     # 16 k-chunks
NE = E // P      # 8 contraction chunks for projections
F16 = mybir.dt.float16
F32 = mybir.dt.float32
Exp = mybir.ActivationFunctionType.Exp
SCALE = float(1.0 / np.sqrt(D))

_prog_cache = {}


def _build():
    nc = bacc.Bacc("TRN2", target_bir_lowering=False, debug=False, num_devices=8)
    XT = nc.dram_tensor("xt", [E, S], F16, kind="ExternalInput")
    WQK = nc.dram_tensor("wqk", [E, 2 * EC], F16, kind="ExternalInput")
    WV = nc.dram_tensor("wv", [E, EC], F16, kind="ExternalInput")
    WO = nc.dram_tensor("wo", [EC, E], F16, kind="ExternalInput")
    BQK = nc.dram_tensor("bqk", [P, 4], F32, kind="ExternalInput")
    BV = nc.dram_tensor("bv", [P, 2], F32, kind="ExternalInput")
    MSK = nc.dram_tensor("msk", [P, 4, TQ], F16, kind="ExternalInput")
    OUT = nc.dram_tensor("out", [E, S], F32, kind="ExternalOutput")

    with tile.TileContext(nc) as tc:
        with (
            tc.tile_pool(name="consts", bufs=1) as consts,
            tc.tile_pool(name="qkp", bufs=1) as qkp,
            tc.tile_pool(name="vp", bufs=1) as vp,
            tc.tile_pool(name="xp", bufs=1) as xp,
            tc.tile_pool(name="ytp", bufs=2) as ytp,
            tc.tile_pool(name="small", bufs=4) as small,
            tc.tile_pool(name="obp", bufs=3) as obp,
            tc.tile_pool(name="etp", bufs=4) as etp,
            tc.tile_pool(name="drp", bufs=4, space="DRAM") as drp,
            tc.tile_pool(name="pgen", bufs=4, space="PSUM") as pgen,
            tc.tile_pool(name="pscore", bufs=2, space="PSUM") as pscore,
        ):
            # ---- input activations first: the xT DMA gates everything ----
            xT = xp.tile([P, NE, S], F16)
            XTr = XT[:].rearrange("(a p) t -> p a t", p=P)
            wv_sb = consts.tile([P, NE, EC], F16)
            nc.sync.dma_start(wv_sb[:], WV[:].rearrange("(a p) c -> p a c", p=P))
            for e in range(NE):
                nc.sync.dma_start(xT[:, e, :], XTr[:, e, :])

            # ---- constants ----
            wqk_sb = consts.tile([P, NE, 2 * EC], F16)
            nc.sync.dma_start(wqk_sb[:], WQK[:].rearrange("(a p) c -> p a c", p=P))
            wo_sb = consts.tile([P, EC // P, E], F16)
            nc.sync.dma_start(wo_sb[:], WO[:].rearrange("(a p) c -> p a c", p=P))
            bqk_sb = consts.tile([P, 4], F32)
            nc.sync.dma_start(bqk_sb[:], BQK[:])
            bv_sb = consts.tile([P, 2], F32)
            nc.sync.dma_start(bv_sb[:], BV[:])
            msk_sb = consts.tile([P, 4, TQ], F16)
            nc.sync.dma_start(msk_sb[:], MSK[:])
            ones_f32 = consts.tile([P, 1], F32)
            nc.vector.memset(ones_f32[:], 1.0)

            qkT = qkp.tile([P, 4, S], F16)
            vaug = vp.tile([P, NKC, NH, D + 1], F16)
            nc.vector.tensor_copy(
                vaug[:, :, :, D : D + 1], ones_f32[:].to_broadcast((P, NKC, NH, 1))
            )

            # ---- emit helpers ----
            def emit_qk_proj(tt, cc):
                pq = pgen.tile([P, TQ], F32, tag="mm")
                for e in range(NE):
                    nc.tensor.matmul(
                        pq[:],
                        wqk_sb[:, e, cc * P : (cc + 1) * P],
                        xT[:, e, tt * TQ : (tt + 1) * TQ],
                        start=(e == 0),
                        stop=(e == NE - 1),
                    )
                nc.vector.tensor_scalar_add(
                    qkT[:, cc, tt * TQ : (tt + 1) * TQ], pq[:], bqk_sb[:, cc : cc + 1]
                )

            def emit_flush(pyt, pbs, yslot, bv_ap):
                nc.vector.tensor_tensor(
                    yslot, pyt[0:D, :], pbs[:], mybir.AluOpType.mult
                )
                nc.vector.tensor_scalar_add(yslot, yslot, bv_ap)

            def emit_outproj(yT, t, eo):
                po = pgen.tile([P, TQ], F32, tag="mm")
                for a in range(EC // P):
                    nc.tensor.matmul(
                        po[:],
                        wo_sb[:, a, eo * P : (eo + 1) * P],
                        yT[:, a, :],
                        start=(a == 0),
                        stop=(a == EC // P - 1),
                    )
                ot = obp.tile([P, TQ], F32)
                nc.vector.tensor_copy(ot[:], po[:])
                nc.sync.dma_start(
                    OUT[eo * P : (eo + 1) * P, t * TQ : (t + 1) * TQ], ot[:]
                )

            from collections import deque

            flush_q = deque()  # normalizations: free pyt PSUM slots, run first
            work_q = deque()   # projection tiles / out-projection tiles

            def pop_deferred(n=1):
                for _ in range(n):
                    if flush_q:
                        flush_q.popleft()()
                    elif work_q:
                        work_q.popleft()()

            def drain_flushes():
                while flush_q:
                    flush_q.popleft()()

            # ---- V projection (row-major, ones column appended) ----
            for c in range(NKC):
                pv = pgen.tile([P, TQ], F32, tag="mm")
                for e in range(NE):
                    nc.tensor.matmul(
                        pv[:, 0:EC],
                        xT[:, e, c * P : (c + 1) * P],
                        wv_sb[:, e, :],
                        start=(e == 0),
                        stop=(e == NE - 1),
                    )
                nc.scalar.copy(
                    vaug[:, c, :, 0:D],
                    pv[:, 0:EC].rearrange("p (h d) -> p h d", d=D),
                )

            # ---- QK projection for t=0; the rest ride the deferred queue ----
            for cc in range(4):
                emit_qk_proj(0, cc)

            # ---- attention, chunk-pipelined, head-pair packed ----
            for t in range(NT):
                if t + 1 < NT:
                    for cc in range(4):
                        work_q.append(lambda tt=t + 1, cc=cc: emit_qk_proj(tt, cc))
                yT = ytp.tile([P, EC // P, TQ], F16)
                for pair in range(2):
                    nchunks = 4 * (t + 1)
                    pyt_he = pgen.tile([P, TQ], F32, tag="mm")
                    pyt_ho = pgen.tile([P, TQ], F32, tag="mm")
                    ets = []
                    for c in range(nchunks):
                        psc = pscore.tile([P, 2 * TQ], F32, tag="sc")
                        nc.tensor.matmul(
                            psc[:, 0:TQ],
                            qkT[0:D, 2 + pair, c * P : (c + 1) * P],
                            qkT[0:D, pair, t * TQ : (t + 1) * TQ],
                            start=True,
                            stop=True,
                        )
                        nc.tensor.matmul(
                            psc[:, TQ : 2 * TQ],
                            qkT[D:P, 2 + pair, c * P : (c + 1) * P],
                            qkT[D:P, pair, t * TQ : (t + 1) * TQ],
                            start=True,
                            stop=True,
                        )
                        et = etp.tile([P, 2, TQ], F16, tag="et")
                        nc.scalar.activation(
                            et[:],
                            psc[:].rearrange("p (a q) -> p a q", a=2),
                            Exp,
                            scale=SCALE,
                        )
                        jd = c - 4 * t
                        if jd >= 0:  # diagonal chunk: causal mask
                            nc.vector.tensor_tensor(
                                et[:],
                                et[:],
                                msk_sb[:, jd : jd + 1, :].to_broadcast((P, 2, TQ)),
                                mybir.AluOpType.mult,
                            )
                        nc.tensor.matmul(
                            pyt_he[0 : D + 1, :],
                            vaug[:, c, 2 * pair, :],
                            et[:, 0, :],
                            start=(c == 0),
                            stop=(c == nchunks - 1),
                        )
                        nc.tensor.matmul(
                            pyt_ho[0 : D + 1, :],
                            vaug[:, c, 2 * pair + 1, :],
                            et[:, 1, :],
                            start=(c == 0),
                            stop=(c == nchunks - 1),
                        )
                        pop_deferred()
                    # reciprocals of the softmax sums (DVE, off the PE path)
                    for idx, pyt in ((0, pyt_he), (1, pyt_ho)):
                        h = 2 * pair + idx
                        pbase = D * idx
                        rc_s = small.tile([1, TQ], F32)
                        nc.vector.tensor_copy(rc_s[:], pyt[D : D + 1, :])
                        rc_f = small.tile([1, TQ], F32)
                        nc.vector.reciprocal_approx_fast(rc_f[:], rc_s[:])
                        rc_d = drp.tile([1, TQ], F32)
                        nc.sync.dma_start(rc_d[:], rc_f[:])
                        pbs = small.tile([D, TQ], F32)
                        nc.sync.dma_start(pbs[:], rc_d[:].to_broadcast((D, TQ)))
                        yslot = yT[pbase : pbase + D, pair, :]
                        bv_ap = bv_sb[pbase : pbase + D, pair : pair + 1]
                        flush_q.append(
                            lambda pyt=pyt, pbs=pbs, yslot=yslot, bv_ap=bv_ap: (
                                emit_flush(pyt, pbs, yslot, bv_ap)
                            )
                        )
                    pop_deferred()
                # out-projection for this q-tile rides the queue too
                for eo in range(NE):
                    work_q.append(lambda yT=yT, t=t, eo=eo: emit_outproj(yT, t, eo))
                pop_deferred(2)
            pop_deferred(len(flush_q) + len(work_q))

    nc.compile()
    return nc


def _shard_inputs(x, W_qkv, b_qkv, W_out, b_out):
    """Build the 8 per-core input maps (host-side layout preprocessing)."""
    Wq, Wk, Wv = W_qkv[:, 0:E], W_qkv[:, E : 2 * E], W_qkv[:, 2 * E : 3 * E]
    bq, bk, bv = b_qkv[0:E], b_qkv[E : 2 * E], b_qkv[2 * E : 3 * E]

    # causal mask for the 4 diagonal 128-chunks of a 512-wide q-tile
    p = np.arange(P)[:, None, None]
    j = np.arange(4)[None, :, None]
    f = np.arange(TQ)[None, None, :]
    msk = (p + P * j <= f).astype(np.float16)

    in_maps = []
    for c in range(8):
        b, hg = c // 4, c % 4
        cs = slice(hg * EC, (hg + 1) * EC)
        in_maps.append(
            {
                "xt": np.ascontiguousarray(x[b].T.astype(np.float16)),
                "wqk": np.ascontiguousarray(
                    np.concatenate([Wq[:, cs], Wk[:, cs]], axis=1).astype(np.float16)
                ),
                "wv": np.ascontiguousarray(Wv[:, cs].astype(np.float16)),
                "wo": np.ascontiguousarray(W_out[cs, :].astype(np.float16)),
                "bqk": np.ascontiguousarray(
                    np.concatenate([bq[cs], bk[cs]]).reshape(4, P).T
                ),
                "bv": np.ascontiguousarray(bv[cs].reshape(2, P).T),
                "msk": msk,
            }
        )
    return in_maps


def _run(inputs, trace=False):
    x = np.asarray(inputs["x"], dtype=np.float32)
    W_qkv = np.asarray(inputs["W_qkv"], dtype=np.float32)
    b_qkv = np.asarray(inputs["b_qkv"], dtype=np.float32)
    W_out = np.asarray(inputs["W_out"], dtype=np.float32)
    b_out = np.asarray(inputs["b_out"], dtype=np.float32)

    if "prog" not in _prog_cache:
        _prog_cache["prog"] = _build()
    nc = _prog_cache["prog"]

    in_maps = _shard_inputs(x, W_qkv, b_qkv, W_out, b_out)
    res = run_bass_kernel_spmd(nc, in_maps, core_ids=list(range(8)), trace=trace)

    out = np.zeros((B, S, E), dtype=np.float64)
    for c in range(8):
        out[c // 4] += res.results[c]["out"].astype(np.float64).T
    out += b_out.astype(np.float64)
    return out.astype(np.float32), res


def kernel(**inputs) -> np.ndarray:
    y, _ = _run(inputs, trace=False)
    return y


# revision 11
# speedup vs baseline: 1.5914x; 1.0209x over previous
"""Causal self-attention (B=2, S=2048, E=1024, H=16, D=64) on 8 TRN2 NeuronCores.

Sharding: core c handles batch b = c//4 and head group hg = c%4 (4 heads).
Each core computes q/k/v projections for its heads, causal attention, and a
row-slice of the output projection; the host sums the 4 partial outputs per
batch and adds b_out.

Matmul operands are fp16 (full-rate PE + fast weight load); accumulation is
fp32 in PSUM. Layouts put every contraction on SBUF partitions:
  qkT   [128, 4, 2048]   Q cols (head pairs 01|23) then K cols, x tokens
  vaug  [128, 16, 4, 65] per tok-chunk, per head, [v | 1] (ones col -> softmax
                         sums appear as row 64 of the y-matmul output)
  yT    [128, 2, 512]    normalized head outputs packed for the out-projection
  outT  [1024, 2048]     partial (y @ W_out).T

Attention is pipelined per 128-token k-chunk: the two heads of a pair are
packed into one [128,1024] scores PSUM tile (even head -> PE rows 0-63, odd
head -> rows 64-127, running concurrently), exp'd in one ScalarE call, then
immediately consumed by the y-matmuls. Softmax normalization, out-projection
and the remaining QK projection tiles ride a deferred-work queue that drains
one item per chunk into the PE stream's ACT-wait gaps, keeping the PE dense
(and the HAM clock warm).
"""

import numpy as np

import concourse.bacc as bacc
import concourse.tile as tile
import concourse.mybir as mybir
from concourse.bass_utils import run_bass_kernel_spmd

B, S, E, H, D = 2, 2048, 1024, 16, 64
NH = 4           # heads per core
EC = NH * D      # 256 embedding cols per core
P = 128
TQ = 512         # q-tile (matmul free dim)
NT = S // TQ     # 4 q-tiles
NKC = S // P     # 16 k-chunks
NE = E // P      # 8 contraction chunks for projections
F16 = mybir.dt.float16
F32 = mybir.dt.float32
Exp = mybir.ActivationFunctionType.Exp
SCALE = float(1.0 / np.sqrt(D))

_prog_cache = {}


def _build():
    nc = bacc.Bacc("TRN2", target_bir_lowering=False, debug=False, num_devices=8)
    XT = nc.dram_tensor("xt", [E, S], F16, kind="ExternalInput")
    WQK = nc.dram_tensor("wqk", [E, 2 * EC], F16, kind="ExternalInput")
    WV = nc.dram_tensor("wv", [E, EC], F16, kind="ExternalInput")
    WO = nc.dram_tensor("wo", [EC, E], F16, kind="ExternalInput")
    BQK = nc.dram_tensor("bqk", [P, 4], F32, kind="ExternalInput")
    BV = nc.dram_tensor("bv", [P, 2], F32, kind="ExternalInput")
    MSK = nc.dram_tensor("msk", [P, 4, TQ], F16, kind="ExternalInput")
    OUT = nc.dram_tensor("out", [E, S], F32, kind="ExternalOutput")

    with tile.TileContext(nc) as tc:
        with (
            tc.tile_pool(name="consts", bufs=1) as consts,
            tc.tile_pool(name="qkp", bufs=1) as qkp,
            tc.tile_pool(name="vp", bufs=1) as vp,
            tc.tile_pool(name="xp", bufs=1) as xp,
            tc.tile_pool(name="ytp", bufs=2) as ytp,
            tc.tile_pool(name="small", bufs=4) as small,
            tc.tile_pool(name="obp", bufs=3) as obp,
            tc.tile_pool(name="etp", bufs=4) as etp,
            tc.tile_pool(name="pgen", bufs=4, space="PSUM") as pgen,
            tc.tile_pool(name="pscore", bufs=2, space="PSUM") as pscore,
        ):
            # ---- input activations first: the xT DMA gates everything ----
            xT = xp.tile([P, NE, S], F16)
            XTr = XT[:].rearrange("(a p) t -> p a t", p=P)
            wv_sb = consts.tile([P, NE, EC], F16)
            nc.sync.dma_start(wv_sb[:], WV[:].rearrange("(a p) c -> p a c", p=P))
            for e in range(NE):
                dma_eng = nc.sync if e % 2 == 0 else nc.gpsimd
                dma_eng.dma_start(xT[:, e, :], XTr[:, e, :])

            # ---- constants ----
            wqk_sb = consts.tile([P, NE, 2 * EC], F16)
            nc.sync.dma_start(wqk_sb[:], WQK[:].rearrange("(a p) c -> p a c", p=P))
            wo_sb = consts.tile([P, EC // P, E], F16)
            nc.sync.dma_start(wo_sb[:], WO[:].rearrange("(a p) c -> p a c", p=P))
            bqk_sb = consts.tile([P, 4], F32)
            nc.sync.dma_start(bqk_sb[:], BQK[:])
            bv_sb = consts.tile([P, 2], F32)
            nc.sync.dma_start(bv_sb[:], BV[:])
            msk_sb = consts.tile([P, 4, TQ], F16)
            nc.sync.dma_start(msk_sb[:], MSK[:])
            ones_f32 = consts.tile([P, 1], F32)
            nc.vector.memset(ones_f32[:], 1.0)
            ones_16 = consts.tile([1, D], F16)
            nc.vector.tensor_copy(ones_16[:], ones_f32[0:1, :].to_broadcast((1, D)))

            qkT = qkp.tile([P, 4, S], F16)
            vaug = vp.tile([P, NKC, NH, D + 1], F16)
            nc.vector.tensor_copy(
                vaug[:, :, :, D : D + 1], ones_f32[:].to_broadcast((P, NKC, NH, 1))
            )

            # ---- emit helpers ----
            def emit_qk_proj(tt, cc):
                pq = pgen.tile([P, TQ], F32, tag="mm")
                for e in range(NE):
                    nc.tensor.matmul(
                        pq[:],
                        wqk_sb[:, e, cc * P : (cc + 1) * P],
                        xT[:, e, tt * TQ : (tt + 1) * TQ],
                        start=(e == 0),
                        stop=(e == NE - 1),
                    )
                nc.vector.tensor_scalar_add(
                    qkT[:, cc, tt * TQ : (tt + 1) * TQ], pq[:], bqk_sb[:, cc : cc + 1]
                )

            def emit_flush(pyt, rc_r, yslot, bv_ap):
                pb = pgen.tile([P, TQ], F32, tag="mm")
                nc.tensor.matmul(
                    pb[0:D, :], ones_16[:], rc_r[:], start=True, stop=True
                )
                pbs = small.tile([D, TQ], F32)
                nc.vector.tensor_copy(pbs[:], pb[0:D, :])
                nc.vector.tensor_tensor(
                    yslot, pyt[0:D, :], pbs[:], mybir.AluOpType.mult
                )
                nc.vector.tensor_scalar_add(yslot, yslot, bv_ap)

            def emit_outproj(yT, t, eo):
                po = pgen.tile([P, TQ], F32, tag="mm")
                for a in range(EC // P):
                    nc.tensor.matmul(
                        po[:],
                        wo_sb[:, a, eo * P : (eo + 1) * P],
                        yT[:, a, :],
                        start=(a == 0),
                        stop=(a == EC // P - 1),
                    )
                ot = obp.tile([P, TQ], F32)
                nc.vector.tensor_copy(ot[:], po[:])
                nc.sync.dma_start(
                    OUT[eo * P : (eo + 1) * P, t * TQ : (t + 1) * TQ], ot[:]
                )

            from collections import deque

            flush_q = deque()  # normalizations: free pyt PSUM slots, run first
            work_q = deque()   # projection tiles / out-projection tiles

            def pop_deferred(n=1, work_ok=True):
                for _ in range(n):
                    if flush_q:
                        flush_q.popleft()()
                    elif work_q and work_ok:
                        work_q.popleft()()

            def drain_flushes():
                while flush_q:
                    flush_q.popleft()()

            # ---- V projection (row-major, ones column appended) ----
            for c in range(NKC):
                pv = pgen.tile([P, TQ], F32, tag="mm")
                for e in range(NE):
                    nc.tensor.matmul(
                        pv[:, 0:EC],
                        xT[:, e, c * P : (c + 1) * P],
                        wv_sb[:, e, :],
                        start=(e == 0),
                        stop=(e == NE - 1),
                    )
                nc.scalar.copy(
                    vaug[:, c, :, 0:D],
                    pv[:, 0:EC].rearrange("p (h d) -> p h d", d=D),
                )

            # ---- QK projection for t=0; the rest ride the deferred queue ----
            for cc in range(4):
                emit_qk_proj(0, cc)

            # ---- attention, chunk-pipelined, head-pair packed ----
            for t in range(NT):
                if t + 1 < NT:
                    for cc in range(4):
                        work_q.append(lambda tt=t + 1, cc=cc: emit_qk_proj(tt, cc))
                yT = ytp.tile([P, EC // P, TQ], F16)
                for pair in range(2):
                    nchunks = 4 * (t + 1)
                    pyt_he = pgen.tile([P, TQ], F32, tag="mm")
                    pyt_ho = pgen.tile([P, TQ], F32, tag="mm")
                    ets = []
                    for c in range(nchunks):
                        psc = pscore.tile([P, 2 * TQ], F32, tag="sc")
                        nc.tensor.matmul(
                            psc[:, 0:TQ],
                            qkT[0:D, 2 + pair, c * P : (c + 1) * P],
                            qkT[0:D, pair, t * TQ : (t + 1) * TQ],
                            start=True,
                            stop=True,
                        )
                        nc.tensor.matmul(
                            psc[:, TQ : 2 * TQ],
                            qkT[D:P, 2 + pair, c * P : (c + 1) * P],
                            qkT[D:P, pair, t * TQ : (t + 1) * TQ],
                            start=True,
                            stop=True,
                        )
                        et = etp.tile([P, 2, TQ], F16, tag="et")
                        nc.scalar.activation(
                            et[:],
                            psc[:].rearrange("p (a q) -> p a q", a=2),
                            Exp,
                            scale=SCALE,
                        )
                        jd = c - 4 * t
                        if jd >= 0:  # diagonal chunk: causal mask
                            nc.vector.tensor_tensor(
                                et[:],
                                et[:],
                                msk_sb[:, jd : jd + 1, :].to_broadcast((P, 2, TQ)),
                                mybir.AluOpType.mult,
                            )
                        nc.tensor.matmul(
                            pyt_he[0 : D + 1, :],
                            vaug[:, c, 2 * pair, :],
                            et[:, 0, :],
                            start=(c == 0),
                            stop=(c == nchunks - 1),
                        )
                        nc.tensor.matmul(
                            pyt_ho[0 : D + 1, :],
                            vaug[:, c, 2 * pair + 1, :],
                            et[:, 1, :],
                            start=(c == 0),
                            stop=(c == nchunks - 1),
                        )
                        pop_deferred(work_ok=(c >= 2))
                    # reciprocals of the softmax sums (DVE, off the PE path)
                    for idx, pyt in ((0, pyt_he), (1, pyt_ho)):
                        h = 2 * pair + idx
                        pbase = D * idx
                        rc_s = small.tile([1, TQ], F32)
                        nc.vector.tensor_copy(rc_s[:], pyt[D : D + 1, :])
                        rc_f = small.tile([1, TQ], F32)
                        nc.vector.reciprocal_approx_fast(rc_f[:], rc_s[:])
                        rc_r = small.tile([1, TQ], F16)
                        nc.vector.tensor_copy(rc_r[:], rc_f[:])
                        yslot = yT[pbase : pbase + D, pair, :]
                        bv_ap = bv_sb[pbase : pbase + D, pair : pair + 1]
                        flush_q.append(
                            lambda pyt=pyt, rc_r=rc_r, yslot=yslot, bv_ap=bv_ap: (
                                emit_flush(pyt, rc_r, yslot, bv_ap)
                            )
                        )
                    pop_deferred()
                # out-projection for this q-tile rides the queue too
                for eo in range(NE):
                    work_q.append(lambda yT=yT, t=t, eo=eo: emit_outproj(yT, t, eo))
            pop_deferred(len(flush_q) + len(work_q))

    nc.compile()
    return nc


def _shard_inputs(x, W_qkv, b_qkv, W_out, b_out):
    """Build the 8 per-core input maps (host-side layout preprocessing)."""
    Wq, Wk, Wv = W_qkv[:, 0:E], W_qkv[:, E : 2 * E], W_qkv[:, 2 * E : 3 * E]
    bq, bk, bv = b_qkv[0:E], b_qkv[E : 2 * E], b_qkv[2 * E : 3 * E]

    # causal mask for the 4 diagonal 128-chunks of a 512-wide q-tile
    p = np.arange(P)[:, None, None]
    j = np.arange(4)[None, :, None]
    f = np.arange(TQ)[None, None, :]
    msk = (p + P * j <= f).astype(np.float16)

    in_maps = []
    for c in range(8):
        b, hg = c // 4, c % 4
        cs = slice(hg * EC, (hg + 1) * EC)
        in_maps.append(
            {
                "xt": np.ascontiguousarray(x[b].T.astype(np.float16)),
                "wqk": np.ascontiguousarray(
                    np.concatenate([Wq[:, cs], Wk[:, cs]], axis=1).astype(np.float16)
                ),
                "wv": np.ascontiguousarray(Wv[:, cs].astype(np.float16)),
                "wo": np.ascontiguousarray(W_out[cs, :].astype(np.float16)),
                "bqk": np.ascontiguousarray(
                    np.concatenate([bq[cs], bk[cs]]).reshape(4, P).T
                ),
                "bv": np.ascontiguousarray(bv[cs].reshape(2, P).T),
                "msk": msk,
            }
        )
    return in_maps


def _run(inputs, trace=False):
    x = np.asarray(inputs["x"], dtype=np.float32)
    W_qkv = np.asarray(inputs["W_qkv"], dtype=np.float32)
    b_qkv = np.asarray(inputs["b_qkv"], dtype=np.float32)
    W_out = np.asarray(inputs["W_out"], dtype=np.float32)
    b_out = np.asarray(inputs["b_out"], dtype=np.float32)

    if "prog" not in _prog_cache:
        _prog_cache["prog"] = _build()
    nc = _prog_cache["prog"]

    in_maps = _shard_inputs(x, W_qkv, b_qkv, W_out, b_out)
    res = run_bass_kernel_spmd(nc, in_maps, core_ids=list(range(8)), trace=trace)

    out = np.zeros((B, S, E), dtype=np.float64)
    for c in range(8):
        out[c // 4] += res.results[c]["out"].astype(np.float64).T
    out += b_out.astype(np.float64)
    return out.astype(np.float32), res


def kernel(**inputs) -> np.ndarray:
    y, _ = _run(inputs, trace=False)
    return y
